# revision 1
# baseline (speedup 1.0000x reference)
"""Distributed Trainium2 kernel for nn_Attention_25228637897408.

GQA attention (B=1, T=2048, D=2048, NH=16, NKV=4, HD=128) with RoPE,
per-head rms_norm, skip-gate blend of k/v, v_bias, causal softmax and
output projection, tensor-parallel over heads on 8 NeuronCores.

Per-core work (core c):
  - q-heads {2c, 2c+1}, kv-head c//2.
  - skip blend done ONCE on activations: stb = x + (g/(1-g))*skip (DVE
    scalar_tensor_tensor); k/v projections contract stb against
    (1-g)-scaled weights -> halves the k/v matmul count.
  - phase 1 streams x/skip in 1024-token halves; each projection unit
    is a dense 32-MM burst whose raw result is copied straight into the
    persistent kT/qT tiles; rms_norm+RoPE epilogues are batched per
    1024-token row and interleaved into later MM bursts so the PE never
    starves (keeps the HAM clock warm).
  - partition reductions (sum-of-squares, softmax denominator) are
    single ones-column matmuls into [1,512] PSUM rows; 1/x and
    1/sqrt(x) via exp(-ln(x)) on ACT: the whole kernel uses one
    activation table set (natural_log_exp_and_others).
  - attention per 128-k-tile: scores -> exp -> y/l accumulate, emitted
    software-pipelined (scores(kt+1) ahead of y/l(kt)) so exp latency
    hides under PE work; causal mask added on the PE via identity x
    mask matmul; exp with per-head scale=gain^2/sqrt(HD) and
    bias=-gain^2*sqrt(HD).
  - AllToAll (gpsimd stays free of DMA work so triggers fire promptly)
    redistributes y; the output projection is split into an h0-block
    pass that overlaps A2A(h1)'s flight and an h1-block pass (8 PSUM
    banks held across the passes).
Host side only reshapes/transposes/casts and slices inputs; all value
computation (sigmoid, blending, norms, softmax, matmuls) is on device.
"""

import sys

sys.path.insert(0, "/opt/trn_rl_repo")

import numpy as np
import ml_dtypes

import concourse.bass as bass
import concourse.mybir as mybir
import concourse.tile as tile
from concourse import bacc
from concourse.bass_utils import run_bass_kernel_spmd

BF16 = ml_dtypes.bfloat16

T = 2048
D = 2048
NH = 16
NKV = 4
HD = 128
REP = NH // NKV
NCORES = 8
HQ = NH // NCORES  # q heads per core = 2
ROPE_BASE = 10000.0
EPS = float(np.finfo(np.float32).eps)
MASK_VAL = -1.0e5

dt = mybir.dt
AF = mybir.ActivationFunctionType
ALU = mybir.AluOpType


def _bf(x):
    return np.ascontiguousarray(np.asarray(x, dtype=np.float32)).astype(BF16)


def build_graph(t=T):
    """Build the SPMD graph (identical on all cores). t parametrizes the
    sequence length for simulator-sized testing."""
    assert t % 1024 == 0
    n_ch = t // 512  # 512-wide token chunks
    n_kt = t // 128  # 128-row tiles along T
    kpc = n_kt // n_ch  # k-tiles per chunk = 4
    rows = t // NCORES  # output rows per core
    n_dt = D // 128  # tiles along D contraction = 16
    HW = 1024  # phase-1 half width
    n_half = t // HW

    nc = bacc.Bacc(None, target_bir_lowering=False)

    xT_d = nc.declare_dram_parameter("xT", [D, t], dt.bfloat16, isOutput=False)
    skT_d = nc.declare_dram_parameter("skipT", [D, t], dt.bfloat16, isOutput=False)
    wqT_d = nc.declare_dram_parameter("wqT", [D, HQ * HD], dt.bfloat16, isOutput=False)
    wkT_d = nc.declare_dram_parameter("wkT", [D, HD], dt.bfloat16, isOutput=False)
    wvT_d = nc.declare_dram_parameter("wvT", [D, HD], dt.bfloat16, isOutput=False)
    wpT_d = nc.declare_dram_parameter("wprojT", [D, D], dt.bfloat16, isOutput=False)
    qkg_d = nc.declare_dram_parameter("qkg", [1, HQ], dt.float32, isOutput=False)
    lns_d = nc.declare_dram_parameter("lns", [1, 1], dt.float32, isOutput=False)
    vb_d = nc.declare_dram_parameter("vbias", [1, HD], dt.float32, isOutput=False)
    cosF_d = nc.declare_dram_parameter("cosF", [HD, t], dt.bfloat16, isOutput=False)
    sinF_d = nc.declare_dram_parameter("sinF", [HD, t], dt.bfloat16, isOutput=False)
    mask_d = nc.declare_dram_parameter("masks", [128, kpc * 512], dt.bfloat16, isOutput=False)
    id_d = nc.declare_dram_parameter("ident", [128, 128], dt.bfloat16, isOutput=False)
    out_d = nc.declare_dram_parameter("out", [rows, D], dt.bfloat16, isOutput=True)

    with tile.TileContext(nc) as tc:
        with (
            tc.tile_pool(name="consts", bufs=1) as cp,
            tc.tile_pool(name="dram", bufs=1, space="DRAM") as dp,
        ):
            fillp = tc.alloc_tile_pool(name="fill_ps", bufs=1, space="PSUM")

            def pe_filler(n):
                """Dependency-chained dummy matmuls: keeps the PE HAM-warm
                through the initial DMA window."""
                fps = fillp.tile([128, 512], dt.float32, tag="fill")
                for i in range(n):
                    nc.tensor.matmul(fps[:], lhsT=ident[:], rhs=mask[:, 0:512], start=(i == 0), stop=(i == n - 1))

            # ---- constants and small scalars ----
            cosF = cp.tile([128, t], dt.bfloat16, tag="cosF")
            sinF = cp.tile([128, t], dt.bfloat16, tag="sinF")
            mask = cp.tile([128, kpc * 512], dt.bfloat16, tag="mask")
            ident = cp.tile([128, 128], dt.bfloat16, tag="ident")
            nc.sync.dma_start(out=mask[:], in_=mask_d[:])
            nc.sync.dma_start(out=ident[:], in_=id_d[:])
            pe_filler(95)

            ones_col = cp.tile([128, 1], dt.bfloat16, tag="ones_col")
            nc.gpsimd.memset(ones_col[:], 1.0)
            onef_row = cp.tile([1, 128], dt.float32, tag="onef_row")
            nc.gpsimd.memset(onef_row[:], 1.0)
            epsb = cp.tile([1, 1], dt.float32, tag="epsb")
            nc.gpsimd.memset(epsb[:], EPS)

            qkg = cp.tile([1, HQ], dt.float32, tag="qkg")
            lns = cp.tile([1, 1], dt.float32, tag="lns")
            vb = cp.tile([1, HD], dt.float32, tag="vb")
            nc.sync.dma_start(out=qkg[:], in_=qkg_d[:])
            nc.sync.dma_start(out=lns[:], in_=lns_d[:])
            nc.sync.dma_start(out=vb[:], in_=vb_d[:])

            # weights (sync/HWDGE so the gpsimd Q7 stays free)
            wkb = cp.tile([128, n_dt * HD], dt.bfloat16, tag="wkb")
            wvb = cp.tile([128, n_dt * HD], dt.bfloat16, tag="wvb")
            wq_sb = cp.tile([128, n_dt * HQ * HD], dt.bfloat16, tag="wq_sb")
            wraw = tc.alloc_tile_pool(name="wraw", bufs=1)
            wk_sb = wraw.tile([128, n_dt * HD], dt.bfloat16, tag="wk_sb")
            wv_sb = wraw.tile([128, n_dt * HD], dt.bfloat16, tag="wv_sb")
            # wq first (the q units run first); wk/wv/cos/sin can trail the
            # first x groups — they are emitted inside the phase-1 loop via
            # late_weight_dmas so x wins the ring-FIFO race.
            nc.sync.dma_start(
                out=wq_sb[:].rearrange("p (k f) -> p k f", k=n_dt),
                in_=wqT_d[:].rearrange("(k p) f -> p k f", p=128),
            )

            def late_weight_dmas():
                nc.sync.dma_start(
                    out=wk_sb[:].rearrange("p (k f) -> p k f", k=n_dt),
                    in_=wkT_d[:].rearrange("(k p) f -> p k f", p=128),
                )
                nc.scalar.dma_start(
                    out=wv_sb[:].rearrange("p (k f) -> p k f", k=n_dt),
                    in_=wvT_d[:].rearrange("(k p) f -> p k f", p=128),
                )
                nc.scalar.dma_start(out=cosF[:], in_=cosF_d[:])
                nc.scalar.dma_start(out=sinF[:], in_=sinF_d[:])
                nc.vector.tensor_scalar_mul(wkb[:], wk_sb[:], omg128[:, 0:1])
                nc.vector.tensor_scalar_mul(wvb[:], wv_sb[:], omg128[:, 0:1])

            # device scalars: g = sigmoid(0.1*lns) via exp so the only ACT
            # table set ever loaded is natural_log_exp_and_others.
            emx = cp.tile([1, 1], dt.float32, tag="emx")
            nc.scalar.activation(emx[:], lns[:], AF.Exp, scale=-0.1)
            onep = cp.tile([1, 1], dt.float32, tag="onep")
            nc.vector.tensor_scalar_add(onep[:], emx[:], 1.0)
            g = cp.tile([1, 1], dt.float32, tag="g")
            nc.vector.reciprocal(g[:], onep[:])  # sigmoid
            omg = cp.tile([1, 1], dt.float32, tag="omg")
            nc.scalar.activation(omg[:], g[:], AF.Copy, bias=1.0, scale=-1.0)  # 1-g
            romg = cp.tile([1, 1], dt.float32, tag="romg")
            nc.vector.reciprocal(romg[:], omg[:])
            rblend = cp.tile([1, 1], dt.float32, tag="rblend")
            nc.vector.tensor_tensor(rblend[:], g[:], romg[:], ALU.mult)  # g/(1-g)
            gainsq = cp.tile([1, HQ], dt.float32, tag="gainsq")
            nc.vector.tensor_mul(gainsq[:], qkg[:], qkg[:])
            # scalar pack: [negC(2), gsc(2), omg, lns, rblend, rgsc(2)] -> 9
            pack = cp.tile([1, 9], dt.float32, tag="pack")
            nc.scalar.activation(pack[:, 0:HQ], gainsq[:], AF.Copy, scale=-float(np.sqrt(HD)))
            nc.scalar.activation(pack[:, 2:4], gainsq[:], AF.Copy, scale=float(1.0 / np.sqrt(HD)))
            nc.vector.tensor_scalar_add(pack[:, 2:4], pack[:, 2:4], 1e-30)
            nc.vector.tensor_copy(pack[:, 4:5], omg[:])
            nc.vector.tensor_copy(pack[:, 5:6], lns[:])
            nc.vector.tensor_copy(pack[:, 6:7], rblend[:])
            nc.vector.reciprocal(pack[:, 7:9], pack[:, 2:4])
            with tc.tile_pool(name="bc_ps", bufs=1, space="PSUM") as bcp:
                pk_ps = bcp.tile([128, 9], dt.float32, tag="pk_ps")
                nc.tensor.matmul(pk_ps[:], lhsT=onef_row[:], rhs=pack[:], start=True, stop=True)
                sc128 = cp.tile([128, 9], dt.float32, tag="sc128")
                nc.vector.tensor_copy(sc128[:], pk_ps[:])
            negC = sc128[:, 0:2]
            gsc = sc128[:, 2:4]
            omg128 = sc128[:, 4:5]
            lns128 = sc128[:, 5:6]
            rbl128 = sc128[:, 6:7]
            rgsc = sc128[:, 7:9]

            # per-head identity * (1/gsc_h) for mask add
            idg = []
            for h in range(HQ):
                t_ = cp.tile([128, 128], dt.bfloat16, tag=f"idg{h}")
                nc.vector.tensor_scalar_mul(t_[:], ident[:], rgsc[:, h : h + 1])
                idg.append(t_)

            # scaled v_bias (1-g)*v_bias, transposed to [128,1] via PE
            vbs = cp.tile([1, HD], dt.float32, tag="vbs")
            nc.vector.tensor_scalar_mul(vbs[:], vb[:], omg[:, 0:1])
            vbsT = cp.tile([128, 1], dt.float32, tag="vbsT")
            with tc.tile_pool(name="bc2_ps", bufs=1, space="PSUM") as bcp2:
                vb_ps = bcp2.tile([128, 1], dt.float32, tag="vb_ps")
                nc.tensor.matmul(vb_ps[:], lhsT=vbs[:], rhs=onef_row[:, 0:1], start=True, stop=True)
                nc.vector.tensor_copy(vbsT[:], vb_ps[:])

            # ---- persistent activations (raw proj, then normed in place) ----
            kT = cp.tile([128, t], dt.bfloat16, tag="kT")
            vT_sb = cp.tile([128, t], dt.bfloat16, tag="vT_sb")
            vnat = cp.tile([128, t], dt.bfloat16, tag="vnat")
            qT = cp.tile([128, HQ * t], dt.bfloat16, tag="qT")

            # ---- phase 1 ----
            fillp.release()  # filler already drained during the DMA window
            with (
                tc.tile_pool(name="xin", bufs=7) as xp,
                tc.tile_pool(name="skin", bufs=3) as skp,
                tc.tile_pool(name="stbp", bufs=16) as sbp,
                tc.tile_pool(name="p1s", bufs=2) as sp,
                tc.tile_pool(name="p1ps", bufs=4, space="PSUM") as psp,
                tc.tile_pool(name="p1row", bufs=2, space="PSUM") as rowp,
                tc.tile_pool(name="p1rb", bufs=1, space="PSUM") as rbp,
                tc.tile_pool(name="vtrp", bufs=1, space="PSUM") as vtrp,
            ):
                def row_epi_part1(raw, hs):
                    """ACT-heavy front of the rms_norm epilogue: squares,
                    partition row-sums, 1/sqrt via ln+exp."""
                    sq = sp.tile([128, HW], dt.bfloat16, tag="sq")
                    nc.scalar.square(sq[:], raw)
                    rsqs = []
                    for col in range(HW // 512):
                        lo = 512 * col
                        ssq = rowp.tile([1, 512], dt.float32, tag="ssq")
                        nc.tensor.matmul(ssq[:], lhsT=ones_col[:], rhs=sq[:, lo : lo + 512], start=True, stop=True)
                        lnr = sp.tile([1, 512], dt.float32, tag="lnr")
                        nc.scalar.activation(lnr[:], ssq[:], AF.Ln, bias=epsb[:, 0:1], scale=1.0 / HD)
                        rsq = sp.tile([1, 512], dt.float32, tag=f"rsq{col}")
                        nc.scalar.activation(rsq[:], lnr[:], AF.Exp, scale=-0.5)
                        rsqs.append(rsq)
                    return (raw, hs, rsqs)

                def row_epi_part2(raw, hs, rsqs):
                    """PE broadcast + normalize + RoPE; emitted a burst after
                    part1 so the ACT chain has already drained."""
                    qh = sp.tile([128, HW], dt.bfloat16, tag="qh")
                    for col in range(HW // 512):
                        lo = 512 * col
                        rb = rbp.tile([128, 512], dt.float32, tag="rb")
                        nc.tensor.matmul(rb[:], lhsT=onef_row[:], rhs=rsqs[col][:], start=True, stop=True)
                        nc.vector.tensor_mul(qh[:, lo : lo + 512], raw[:, lo : lo + 512], rb[:])
                    qsw = sp.tile([128, HW], dt.bfloat16, tag="qsw")
                    nc.vector.tensor_copy(qsw[0:64, :], qh[64:128, :])
                    nc.vector.tensor_copy(qsw[64:128, :], qh[0:64, :])
                    tsw = sp.tile([128, HW], dt.bfloat16, tag="tsw")
                    nc.vector.tensor_mul(tsw[:], qsw[:], sinF[:, hs : hs + HW])
                    tco = sp.tile([128, HW], dt.bfloat16, tag="tco")
                    nc.vector.tensor_mul(tco[:], qh[:], cosF[:, hs : hs + HW])
                    nc.vector.tensor_add(raw, tco[:], tsw[:])

                def vnat_transpose(grp):
                    vtp = vtrp.tile([128, 512], dt.bfloat16, tag="vtr")
                    for s_ in range(4):
                        kt = 4 * grp + s_
                        nc.tensor.transpose(vtp[:, 128 * s_ : 128 * (s_ + 1)], vT_sb[:, 128 * kt : 128 * (kt + 1)], ident[:])
                    nc.vector.tensor_copy(vnat[:, 512 * grp : 512 * (grp + 1)], vtp[:])

                # deferred work queue: callables interleaved into later bursts
                deferred = []

                GD = 4  # d-tiles per coalesced DMA group
                for hf in range(n_half):
                    hs = HW * hf
                    xg = []
                    stb = []
                    # x first on BOTH rings (q units need only x), then skip
                    for gi in range(n_dt // GD):
                        xx = xp.tile([128, GD * HW], dt.bfloat16, tag="xg")
                        eng = nc.sync if gi % 2 == 0 else nc.scalar
                        eng.dma_start(
                            out=xx[:].rearrange("p (k f) -> p k f", k=GD),
                            in_=xT_d[128 * GD * gi : 128 * GD * (gi + 1), hs : hs + HW].rearrange("(k p) f -> p k f", p=128),
                        )
                        xg.append(xx)
                    if hf == 0:
                        late_weight_dmas()
                    for gi in range(n_dt // GD):
                        ss = skp.tile([128, GD * HW], dt.bfloat16, tag="sg")
                        eng = nc.scalar if gi % 2 == 0 else nc.sync
                        eng.dma_start(
                            out=ss[:].rearrange("p (k f) -> p k f", k=GD),
                            in_=skT_d[128 * GD * gi : 128 * GD * (gi + 1), hs : hs + HW].rearrange("(k p) f -> p k f", p=128),
                        )
                        for kk in range(GD):
                            bl = sbp.tile([128, HW], dt.bfloat16, tag="stb")
                            nc.vector.scalar_tensor_tensor(
                                bl[:], ss[:, kk * HW : (kk + 1) * HW], rbl128[:, 0:1],
                                xg[gi][:, kk * HW : (kk + 1) * HW], ALU.mult, ALU.add,
                            )
                            stb.append(bl)

                    def xs(k, ls):
                        return xg[k // GD][:, (k % GD) * HW + ls.start : (k % GD) * HW + ls.stop]

                    for kind in ("q0", "q1", "k", "v"):
                        for col in range(HW // 512):
                            # interleave one deferred epilogue ahead of the
                            # burst: its ACT chain drained a burst ago, so
                            # its PE ops slot in without stalling the FIFO
                            if deferred:
                                deferred.pop(0)()
                            cs = slice(hs + 512 * col, hs + 512 * (col + 1))
                            ls = slice(512 * col, 512 * (col + 1))
                            ps = psp.tile([128, 512], dt.float32, tag="proj_ps")
                            if kind == "k":
                                dest = kT[:, cs]
                                for k in range(n_dt):
                                    nc.tensor.matmul(ps[:], lhsT=wkb[:, k * HD : (k + 1) * HD], rhs=stb[k][:, ls], start=(k == 0), stop=(k == n_dt - 1))
                            elif kind == "v":
                                dest = vT_sb[:, cs]
                                for k in range(n_dt):
                                    nc.tensor.matmul(ps[:], lhsT=wvb[:, k * HD : (k + 1) * HD], rhs=stb[k][:, ls], start=(k == 0), stop=(k == n_dt - 1))
                            else:
                                h = int(kind[1])
                                dest = qT[:, t * h + hs + 512 * col : t * h + hs + 512 * (col + 1)]
                                for k in range(n_dt):
                                    nc.tensor.matmul(
                                        ps[:],
                                        lhsT=wq_sb[:, k * HQ * HD + h * HD : k * HQ * HD + (h + 1) * HD],
                                        rhs=xs(k, ls),
                                        start=(k == 0),
                                        stop=(k == n_dt - 1),
                                    )
                            if kind == "v":
                                # v epilogue inline: add (1-g)*v_bias
                                nc.vector.tensor_scalar_add(dest, ps[:], vbsT[:, 0:1])
                            else:
                                nc.scalar.activation(dest, ps[:], AF.Copy)
                        # queue each row's epilogue part1 as soon as its
                        # bursts are emitted; part1 re-queues part2 so the
                        # PE broadcast lands a burst after the ACT chain.
                        # vnat transposes run inline after v.
                        if kind == "v":
                            vnat_transpose(2 * hf)
                            vnat_transpose(2 * hf + 1)
                        else:
                            if kind == "k":
                                raw = kT[:, hs : hs + HW]
                            else:
                                h = int(kind[1])
                                raw = qT[:, t * h + hs : t * h + hs + HW]

                            def p1(raw=raw, hs=hs):
                                st_ = row_epi_part1(raw, hs)
                                deferred.append(lambda: row_epi_part2(*st_))

                            deferred.append(p1)

                # drain remaining deferred work
                for d in deferred:
                    d()
                deferred = []
            wraw.release()

            # ---- phase 2: attention ----
            y_in = [dp.tile([NCORES, HD, rows], dt.bfloat16, name=f"y_in{h}", tag=f"y_in{h}") for h in range(HQ)]
            y_out = [dp.tile([NCORES, HD, rows], dt.bfloat16, name=f"y_out{h}", tag=f"y_out{h}") for h in range(HQ)]

            # wproj prefetch (full, both HWDGE rings) during phase 2;
            # one 2 MB coalesced DMA per column block
            prp = tc.alloc_tile_pool(name="pr_s", bufs=4)
            wps = {}
            for n in range(D // 512):
                wp = prp.tile([128, n_dt * 512], dt.bfloat16, name=f"wp{n}", tag="wp")
                eng = nc.sync if n % 2 == 0 else nc.scalar
                eng.dma_start(
                    out=wp[:].rearrange("p (k f) -> p k f", k=n_dt),
                    in_=wpT_d[:, 512 * n : 512 * (n + 1)].rearrange("(k p) f -> p k f", p=128),
                )
                wps[n] = wp

            ytp_ = tc.alloc_tile_pool(name="yt_s", bufs=1)
            yt_blocks = [None] * n_dt
            with (
                tc.tile_pool(name="att_s", bufs=5) as ap_,
                tc.tile_pool(name="acc_s", bufs=2) as ap2,
                tc.tile_pool(name="st_ps", bufs=3, space="PSUM") as stp_,
                tc.tile_pool(name="yl_ps", bufs=2, space="PSUM") as ylp_,
            ):
                def epi_part1(h, c, ytp, accA, accB):
                    # l = ones.T @ (accA + accB); 1/l = exp(-ln(l))
                    lrow_t = stp_.tile([128, 1024], dt.float32, tag="st", name="lrow_t")
                    lrow = lrow_t[0:1, 0:512]
                    nc.tensor.matmul(lrow, lhsT=ones_col[:], rhs=accA[:], start=True, stop=False)
                    nc.tensor.matmul(lrow, lhsT=ones_col[:], rhs=accB[:], start=False, stop=True)
                    lnl = ap_.tile([1, 512], dt.float32, tag="lnl")
                    nc.scalar.activation(lnl[:], lrow, AF.Ln, bias=0.0, scale=1.0)
                    rl = ap_.tile([1, 512], dt.float32, tag="rl")
                    nc.scalar.activation(rl[:], lnl[:], AF.Exp, scale=-1.0)
                    return (h, c, ytp, rl)

                def epi_part2(h, c, ytp, rl):
                    # broadcast 1/l, normalize, ship pieces
                    rb2_t = stp_.tile([128, 1024], dt.float32, tag="st", name="rb2_t")
                    rb2 = rb2_t[:, 0:512]
                    nc.tensor.matmul(rb2, lhsT=onef_row[:], rhs=rl[:], start=True, stop=True)
                    rb2s = ap_.tile([128, 512], dt.float32, tag="rb2s")
                    nc.vector.tensor_copy(rb2s[:], rb2)
                    ysb = ap_.tile([128, 512], dt.bfloat16, tag="ysb")
                    nc.vector.tensor_mul(ysb[:], ytp[:], rb2s[:])
                    for b in range(512 // rows):
                        piece = (512 * c) // rows + b
                        nc.sync.dma_start(
                            out=y_in[h][piece, :, :],
                            in_=ysb[:, rows * b : rows * (b + 1)],
                        )

                prev_epi = None  # (h, c, ytp, acc): chunk awaiting part1
                epi1 = None  # (h, c, ytp, rl): awaiting part2
                for h in range(HQ):
                    for c in range(n_ch):
                        qs = slice(t * h + 512 * c, t * h + 512 * (c + 1))
                        nkts = kpc * (c + 1)
                        ytp = ylp_.tile([128, 512], dt.float32, tag="yt")
                        # two alternating accumulators halve the serial DVE
                        # dependency chain for the softmax denominator
                        accA = ap2.tile([128, 512], dt.bfloat16, tag="accA")
                        accB = ap2.tile([128, 512], dt.bfloat16, tag="accB")
                        pend = []  # [(pp, kts)] awaiting y/acc emission (2-deep)

                        def emit_pend(p, last, ytp=ytp, accA=accA, accB=accB):
                            ppp, kts_ = p
                            for s_, kt_ in enumerate(kts_):
                                pseg = ppp[:, 512 * s_ : 512 * (s_ + 1)]
                                nc.tensor.matmul(ytp[:], lhsT=vnat[:, HD * kt_ : HD * (kt_ + 1)], rhs=pseg, start=(kt_ == 0), stop=(last and kt_ == kts_[-1]))
                                acc = accA if kt_ % 2 == 0 else accB
                                if kt_ < 2:
                                    nc.vector.tensor_copy(acc[:], pseg)
                                else:
                                    nc.vector.tensor_add(acc[:], acc[:], pseg)

                        for pgi in range(nkts // 2):
                            kts = [2 * pgi, 2 * pgi + 1]
                            stp = stp_.tile([128, 1024], dt.float32, tag="st")
                            for s, kt in enumerate(kts):
                                seg = stp[:, 512 * s : 512 * (s + 1)]
                                diag = kt >= kpc * c
                                nc.tensor.matmul(seg, lhsT=kT[:, 128 * kt : 128 * (kt + 1)], rhs=qT[:, qs], start=True, stop=not diag)
                                if diag:
                                    m = kt - kpc * c
                                    nc.tensor.matmul(seg, lhsT=idg[h][:], rhs=mask[:, 512 * m : 512 * (m + 1)], start=False, stop=True)
                            # 2-deep lookahead: y/acc for group i-2 land after
                            # scores of group i, so exp latency is fully hidden
                            if len(pend) >= 2:
                                emit_pend(pend.pop(0), last=False)
                            pp = ap_.tile([128, 1024], dt.bfloat16, tag="pp")
                            nc.scalar.activation(pp[:], stp[:], AF.Exp, bias=negC[:, h : h + 1], scale=gsc[:, h : h + 1])
                            npg = nkts // 2
                            p1_at = min(npg - 2, 3) if npg >= 3 else 0
                            if pgi == p1_at and prev_epi is not None:
                                epi1 = epi_part1(*prev_epi)
                                prev_epi = None
                            elif pgi == p1_at + 1 and epi1 is not None:
                                epi_part2(*epi1)
                                epi1 = None
                            pend.append((pp, kts))
                        while pend:
                            emit_pend(pend.pop(0), last=(len(pend) == 0))
                        prev_epi = (h, c, ytp, accA, accB)
                    # flush last chunk's epilogue before the collective
                    epi_part2(*epi_part1(*prev_epi))
                    prev_epi = None
                    nc.gpsimd.collective_compute(
                        "AllToAll",
                        ALU.bypass,
                        replica_groups=[list(range(NCORES))],
                        ins=[y_in[h].opt()],
                        outs=[y_out[h].opt()],
                    )
                    if h == 0:
                        yb = ytp_.tile([128, NCORES * rows], dt.bfloat16, name="ytall0", tag="ytall0")
                        nc.sync.dma_start(
                            out=yb[:].rearrange("p (j r) -> p j r", j=NCORES),
                            in_=y_out[0][:].rearrange("j p r -> p j r"),
                        )
                        for j in range(NCORES):
                            yt_blocks[2 * j] = yb[:, rows * j : rows * (j + 1)]

            # ---- phase 3: output projection (h0 pass overlaps A2A(h1)) ----
            yb1 = ytp_.tile([128, NCORES * rows], dt.bfloat16, name="ytall1", tag="ytall1")
            nc.sync.dma_start(
                out=yb1[:].rearrange("p (j r) -> p j r", j=NCORES),
                in_=y_out[1][:].rearrange("j p r -> p j r"),
            )
            for j in range(NCORES):
                yt_blocks[2 * j + 1] = yb1[:, rows * j : rows * (j + 1)]

            mb = min(128, rows)
            nb = rows // mb
            tiles3 = [(n, b) for n in range(D // 512) for b in range(nb)]
            with (
                tc.tile_pool(name="pr_ps", bufs=1, space="PSUM") as prps,
                tc.tile_pool(name="pr_out", bufs=2) as prout,
            ):
                opss = {}
                for (n, b) in tiles3:
                    ops = prps.tile([mb, 512], dt.float32, tag=f"ops{n}_{b}")
                    opss[(n, b)] = ops
                    for ai, a in enumerate(range(0, n_dt, 2)):  # h0 blocks
                        nc.tensor.matmul(
                            ops[:],
                            lhsT=yt_blocks[a][:, mb * b : mb * (b + 1)],
                            rhs=wps[n][:, 512 * a : 512 * (a + 1)],
                            start=(ai == 0),
                            stop=False,
                        )
                for (n, b) in tiles3:
                    ops = opss[(n, b)]
                    for ai, a in enumerate(range(1, n_dt, 2)):  # h1 blocks
                        nc.tensor.matmul(
                            ops[:],
                            lhsT=yt_blocks[a][:, mb * b : mb * (b + 1)],
                            rhs=wps[n][:, 512 * a : 512 * (a + 1)],
                            start=False,
                            stop=(ai == n_dt // 2 - 1),
                        )
                    osb = prout.tile([mb, 512], dt.bfloat16, tag="osb")
                    nc.scalar.activation(osb[:], ops[:], AF.Copy, scale=lns128[:mb, 0:1])
                    nc.sync.dma_start(
                        out=out_d[mb * b : mb * (b + 1), 512 * n : 512 * (n + 1)],
                        in_=osb[:],
                    )
            ytp_.release()
            prp.release()
    nc.finalize()
    return nc


def make_tables(t=T):
    pos = np.arange(t, dtype=np.float32)
    inv = 1.0 / (ROPE_BASE ** (np.arange(0, HD, 2, dtype=np.float32) / HD))
    fr = pos[:, None] * inv[None, :]  # [t, 64]
    cos = np.cos(fr).T  # [64, t]
    sin = np.sin(fr).T
    cosF = np.concatenate([cos, cos], axis=0)  # [128, t]
    sinF = np.concatenate([sin, -sin], axis=0)
    return _bf(cosF), _bf(sinF)


def make_masks():
    # mask[p, 512*m + j] = 0 if j >= 128*m + p else MASK_VAL
    p = np.arange(128)[:, None]
    j = np.arange(512)[None, :]
    blocks = [np.where(j >= 128 * m + p, 0.0, MASK_VAL) for m in range(4)]
    return _bf(np.concatenate(blocks, axis=1))


_GRAPH_CACHE = {}
_LAST_IN_MAPS = None


def kernel(x, skip, wq, wk, wv, wproj, qk_g, ln_s, v_bias):
    t = x.shape[1]
    if t not in _GRAPH_CACHE:
        _GRAPH_CACHE[t] = build_graph(t)
    nc = _GRAPH_CACHE[t]

    xT = _bf(x.reshape(t, D).T)
    skT = _bf(skip.reshape(t, D).T)
    wpT = _bf(np.asarray(wproj, np.float32).T)
    cosF, sinF = make_tables(t)
    masks = make_masks()
    ident = _bf(np.eye(128, dtype=np.float32))

    in_maps = []
    for c in range(NCORES):
        kv = c // 2
        in_maps.append(
            {
                "xT": xT,
                "skipT": skT,
                "wqT": _bf(np.asarray(wq, np.float32)[HQ * HD * c : HQ * HD * (c + 1), :].T),
                "wkT": _bf(np.asarray(wk, np.float32)[HD * kv : HD * (kv + 1), :].T),
                "wvT": _bf(np.asarray(wv, np.float32)[HD * kv : HD * (kv + 1), :].T),
                "wprojT": wpT,
                "qkg": np.asarray(qk_g, np.float32)[HQ * c : HQ * (c + 1)].reshape(1, HQ),
                "lns": np.asarray(ln_s, np.float32).reshape(1, 1),
                "vbias": np.asarray(v_bias, np.float32)[kv].reshape(1, HD),
                "cosF": cosF,
                "sinF": sinF,
                "masks": masks,
                "ident": ident,
            }
        )
    global _LAST_IN_MAPS
    _LAST_IN_MAPS = in_maps
    res = run_bass_kernel_spmd(nc, in_maps, list(range(NCORES)))
    out = np.concatenate(
        [np.asarray(res.results[c]["out"], np.float32) for c in range(NCORES)], axis=0
    )
    return out.reshape(1, t, D).astype(np.float32)



# revision 18
# speedup vs baseline: 1.0590x; 1.0590x over previous
"""Distributed Trainium2 kernel for nn_Attention_25228637897408.

GQA attention (B=1, T=2048, D=2048, NH=16, NKV=4, HD=128) with RoPE,
per-head rms_norm, skip-gate blend of k/v, v_bias, causal softmax and
output projection, tensor-parallel over heads on 8 NeuronCores.

Per-core work (core c):
  - q-heads {2c, 2c+1}, kv-head c//2.
  - skip blend done ONCE on activations: stb = x + (g/(1-g))*skip (DVE
    scalar_tensor_tensor); k/v projections contract stb against
    (1-g)-scaled weights -> halves the k/v matmul count.
  - phase 1 streams x/skip in 1024-token halves; each projection unit
    is a dense 32-MM burst whose raw result is copied straight into the
    persistent kT/qT tiles; rms_norm+RoPE epilogues are batched per
    1024-token row and interleaved into later MM bursts so the PE never
    starves (keeps the HAM clock warm).
  - partition reductions (sum-of-squares, softmax denominator) are
    single ones-column matmuls into [1,512] PSUM rows; 1/x and
    1/sqrt(x) via exp(-ln(x)) on ACT: the whole kernel uses one
    activation table set (natural_log_exp_and_others).
  - attention per 128-k-tile: scores -> exp -> y/l accumulate, emitted
    software-pipelined (scores(kt+1) ahead of y/l(kt)) so exp latency
    hides under PE work; causal mask added on the PE via identity x
    mask matmul; exp with per-head scale=gain^2/sqrt(HD) and
    bias=-gain^2*sqrt(HD).
  - AllToAll (gpsimd stays free of DMA work so triggers fire promptly)
    redistributes y; the output projection is split into an h0-block
    pass that overlaps A2A(h1)'s flight and an h1-block pass (8 PSUM
    banks held across the passes).
Host side only reshapes/transposes/casts and slices inputs; all value
computation (sigmoid, blending, norms, softmax, matmuls) is on device.
"""

import sys

sys.path.insert(0, "/opt/trn_rl_repo")

import numpy as np
import ml_dtypes

import concourse.bass as bass
import concourse.mybir as mybir
import concourse.tile as tile
from concourse import bacc
from concourse.bass_utils import run_bass_kernel_spmd

BF16 = ml_dtypes.bfloat16

T = 2048
D = 2048
NH = 16
NKV = 4
HD = 128
REP = NH // NKV
NCORES = 8
HQ = NH // NCORES  # q heads per core = 2
ROPE_BASE = 10000.0
EPS = float(np.finfo(np.float32).eps)
MASK_VAL = -1.0e5

dt = mybir.dt
AF = mybir.ActivationFunctionType
ALU = mybir.AluOpType


def _bf(x):
    return np.ascontiguousarray(np.asarray(x, dtype=np.float32)).astype(BF16)


def build_graph(t=T):
    """Build the SPMD graph (identical on all cores). t parametrizes the
    sequence length for simulator-sized testing."""
    assert t % 1024 == 0
    n_ch = t // 512  # 512-wide token chunks
    n_kt = t // 128  # 128-row tiles along T
    kpc = n_kt // n_ch  # k-tiles per chunk = 4
    rows = t // NCORES  # output rows per core
    n_dt = D // 128  # tiles along D contraction = 16
    HW = 1024  # phase-1 half width
    n_half = t // HW

    nc = bacc.Bacc(None, target_bir_lowering=False)

    xT_d = nc.declare_dram_parameter("xT", [D, t], dt.bfloat16, isOutput=False)
    skT_d = nc.declare_dram_parameter("skipT", [D, t], dt.bfloat16, isOutput=False)
    wqT_d = nc.declare_dram_parameter("wqT", [D, HQ * HD], dt.bfloat16, isOutput=False)
    wkT_d = nc.declare_dram_parameter("wkT", [D, HD], dt.bfloat16, isOutput=False)
    wvT_d = nc.declare_dram_parameter("wvT", [D, HD], dt.bfloat16, isOutput=False)
    wpT_d = nc.declare_dram_parameter("wprojT", [D, D], dt.bfloat16, isOutput=False)
    qkg_d = nc.declare_dram_parameter("qkg", [1, HQ], dt.float32, isOutput=False)
    lns_d = nc.declare_dram_parameter("lns", [1, 1], dt.float32, isOutput=False)
    vb_d = nc.declare_dram_parameter("vbias", [1, HD], dt.float32, isOutput=False)
    cosF_d = nc.declare_dram_parameter("cosF", [HD, t], dt.bfloat16, isOutput=False)
    sinF_d = nc.declare_dram_parameter("sinF", [HD, t], dt.bfloat16, isOutput=False)
    mask_d = nc.declare_dram_parameter("masks", [128, kpc * 512], dt.bfloat16, isOutput=False)
    id_d = nc.declare_dram_parameter("ident", [128, 128], dt.bfloat16, isOutput=False)
    out_d = nc.declare_dram_parameter("out", [rows, D], dt.bfloat16, isOutput=True)

    with tile.TileContext(nc) as tc:
        with (
            tc.tile_pool(name="consts", bufs=1) as cp,
            tc.tile_pool(name="dram", bufs=1, space="DRAM") as dp,
        ):
            fillp = tc.alloc_tile_pool(name="fill_ps", bufs=1, space="PSUM")

            def pe_filler(n):
                """Dependency-chained dummy matmuls: keeps the PE HAM-warm
                through the initial DMA window."""
                fps = fillp.tile([128, 512], dt.float32, tag="fill")
                for i in range(n):
                    nc.tensor.matmul(fps[:], lhsT=ident[:], rhs=mask[:, 0:512], start=(i == 0), stop=(i == n - 1))

            # ---- constants and small scalars ----
            cosF = cp.tile([128, t], dt.bfloat16, tag="cosF")
            sinF = cp.tile([128, t], dt.bfloat16, tag="sinF")
            mask = cp.tile([128, kpc * 512], dt.bfloat16, tag="mask")
            ident = cp.tile([128, 128], dt.bfloat16, tag="ident")
            nc.sync.dma_start(out=mask[:], in_=mask_d[:])
            nc.sync.dma_start(out=ident[:], in_=id_d[:])
            pe_filler(95)

            ones_col = cp.tile([128, 1], dt.bfloat16, tag="ones_col")
            nc.gpsimd.memset(ones_col[:], 1.0)
            onef_row = cp.tile([1, 128], dt.float32, tag="onef_row")
            nc.gpsimd.memset(onef_row[:], 1.0)

            qkg = cp.tile([1, HQ], dt.float32, tag="qkg")
            lns = cp.tile([1, 1], dt.float32, tag="lns")
            vb = cp.tile([1, HD], dt.float32, tag="vb")
            nc.sync.dma_start(out=qkg[:], in_=qkg_d[:])
            nc.sync.dma_start(out=lns[:], in_=lns_d[:])
            nc.sync.dma_start(out=vb[:], in_=vb_d[:])

            # weights (sync/HWDGE so the gpsimd Q7 stays free)
            wkb = cp.tile([128, n_dt * HD], dt.bfloat16, tag="wkb")
            wvb = cp.tile([128, n_dt * HD], dt.bfloat16, tag="wvb")
            wq_sb = cp.tile([128, n_dt * HQ * HD], dt.bfloat16, tag="wq_sb")
            wraw = tc.alloc_tile_pool(name="wraw", bufs=1)
            wk_sb = wraw.tile([128, n_dt * HD], dt.bfloat16, tag="wk_sb")
            wv_sb = wraw.tile([128, n_dt * HD], dt.bfloat16, tag="wv_sb")
            # wq first (the q units run first); wk/wv/cos/sin can trail the
            # first x groups — they are emitted inside the phase-1 loop via
            # late_weight_dmas so x wins the ring-FIFO race.
            nc.sync.dma_start(
                out=wq_sb[:].rearrange("p (k f) -> p k f", k=n_dt),
                in_=wqT_d[:].rearrange("(k p) f -> p k f", p=128),
            )

            def late_weight_dmas():
                nc.sync.dma_start(
                    out=wk_sb[:].rearrange("p (k f) -> p k f", k=n_dt),
                    in_=wkT_d[:].rearrange("(k p) f -> p k f", p=128),
                )
                nc.scalar.dma_start(
                    out=wv_sb[:].rearrange("p (k f) -> p k f", k=n_dt),
                    in_=wvT_d[:].rearrange("(k p) f -> p k f", p=128),
                )
                nc.scalar.dma_start(out=cosF[:], in_=cosF_d[:])
                nc.scalar.dma_start(out=sinF[:], in_=sinF_d[:])
                nc.vector.tensor_scalar_mul(wkb[:], wk_sb[:], omg128[:, 0:1])
                nc.vector.tensor_scalar_mul(wvb[:], wv_sb[:], omg128[:, 0:1])

            # device scalars: g = sigmoid(0.1*lns) via exp so the only ACT
            # table set ever loaded is one containing exp (no Ln anywhere in
            # this kernel -> zero ACT_TABLE_LOAD swaps after the first).
            emx = cp.tile([1, 1], dt.float32, tag="emx")
            nc.scalar.activation(emx[:], lns[:], AF.Exp, scale=-0.1)
            onep = cp.tile([1, 1], dt.float32, tag="onep")
            nc.vector.tensor_scalar_add(onep[:], emx[:], 1.0)
            g = cp.tile([1, 1], dt.float32, tag="g")
            nc.vector.reciprocal(g[:], onep[:])  # sigmoid
            omg = cp.tile([1, 1], dt.float32, tag="omg")
            nc.scalar.activation(omg[:], g[:], AF.Copy, bias=1.0, scale=-1.0)  # 1-g
            romg = cp.tile([1, 1], dt.float32, tag="romg")
            nc.vector.reciprocal(romg[:], omg[:])
            rblend = cp.tile([1, 1], dt.float32, tag="rblend")
            nc.vector.tensor_tensor(rblend[:], g[:], romg[:], ALU.mult)  # g/(1-g)
            gainsq = cp.tile([1, HQ], dt.float32, tag="gainsq")
            nc.vector.tensor_mul(gainsq[:], qkg[:], qkg[:])
            # gain^2/sqrt(HD) per head (folded into qT at write time), and
            # -Cmax = -sqrt(HD)*max_h gain^2 (softmax exp bias; head-indep)
            gsc2 = cp.tile([1, HQ], dt.float32, tag="gsc2")
            nc.vector.tensor_scalar_mul(gsc2[:], gainsq[:], float(1.0 / np.sqrt(HD)))
            gmax2 = cp.tile([1, 1], dt.float32, tag="gmax2")
            nc.vector.tensor_tensor(gmax2[:], gainsq[:, 0:1], gainsq[:, 1:2], ALU.max)
            # scalar pack: [negCmax, omg, lns, rblend] -> 4
            pack = cp.tile([1, 4], dt.float32, tag="pack")
            nc.scalar.activation(pack[:, 0:1], gmax2[:], AF.Copy, scale=-float(np.sqrt(HD)))
            nc.vector.tensor_copy(pack[:, 1:2], omg[:])
            nc.vector.tensor_copy(pack[:, 2:3], lns[:])
            nc.vector.tensor_copy(pack[:, 3:4], rblend[:])
            with tc.tile_pool(name="bc_ps", bufs=1, space="PSUM") as bcp:
                pk_ps = bcp.tile([128, 4], dt.float32, tag="pk_ps")
                nc.tensor.matmul(pk_ps[:], lhsT=onef_row[:], rhs=pack[:], start=True, stop=True)
                sc128 = cp.tile([128, 4], dt.float32, tag="sc128")
                nc.vector.tensor_copy(sc128[:], pk_ps[:])
                # per-unit scale rows for the rms broadcast matmul, placed at
                # partitions 0/32/64 (q0: g0^2/sqrt(HD), q1: g1^2/sqrt(HD),
                # k: 1.0) so lhsT/rhs/out base-partition rules line up with
                # the rsqrt rows living at partitions 0/32/64.
                gsc3 = cp.tile([1, 3], dt.float32, tag="gsc3")
                nc.vector.tensor_copy(gsc3[:, 0:HQ], gsc2[:])
                nc.gpsimd.memset(gsc3[:, 2:3], 1.0)
                gq_ps = bcp.tile([128, 128], dt.float32, tag="gq_ps")
                for u in range(3):
                    nc.tensor.matmul(gq_ps[32 * u : 32 * u + 1, :], lhsT=gsc3[:, u : u + 1], rhs=onef_row[:], start=True, stop=True)
                growq = cp.tile([128, 128], dt.float32, tag="growq")
                for u in range(3):
                    nc.vector.tensor_copy(growq[32 * u : 32 * u + 1, :], gq_ps[32 * u : 32 * u + 1, :])
            negCmax = sc128[:, 0:1]
            omg128 = sc128[:, 1:2]
            lns128 = sc128[:, 2:3]
            rbl128 = sc128[:, 3:4]

            # scaled v_bias (1-g)*v_bias, transposed to [128,1] via PE
            vbs = cp.tile([1, HD], dt.float32, tag="vbs")
            nc.vector.tensor_scalar_mul(vbs[:], vb[:], omg[:, 0:1])
            vbsT = cp.tile([128, 1], dt.float32, tag="vbsT")
            with tc.tile_pool(name="bc2_ps", bufs=1, space="PSUM") as bcp2:
                vb_ps = bcp2.tile([128, 1], dt.float32, tag="vb_ps")
                nc.tensor.matmul(vb_ps[:], lhsT=vbs[:], rhs=onef_row[:, 0:1], start=True, stop=True)
                nc.vector.tensor_copy(vbsT[:], vb_ps[:])

            # ---- persistent activations (raw proj, then normed in place) ----
            kT = cp.tile([128, t], dt.bfloat16, tag="kT")
            vT_sb = cp.tile([128, t], dt.bfloat16, tag="vT_sb")
            vnat = cp.tile([128, t], dt.bfloat16, tag="vnat")
            qT = cp.tile([128, HQ * t], dt.bfloat16, tag="qT")

            # ---- phase 1 ----
            fillp.release()  # filler already drained during the DMA window
            with (
                tc.tile_pool(name="xin", bufs=7) as xp,
                tc.tile_pool(name="skin", bufs=3) as skp,
                tc.tile_pool(name="stbp", bufs=16) as sbp,
                tc.tile_pool(name="p1s", bufs=2) as sp,
                tc.tile_pool(name="p1ps", bufs=4, space="PSUM") as psp,
                tc.tile_pool(name="p1row", bufs=1, space="PSUM") as rowp,
                tc.tile_pool(name="p1rb", bufs=1, space="PSUM") as rbp,
                tc.tile_pool(name="vtrp", bufs=1, space="PSUM") as vtrp,
            ):
                # rms_norm rsqrt via bit-trick ln + exp-table + 2 Newton steps
                # (keeps the ACT table on the exp set: no Ln -> no table swaps)
                LN2 = float(np.log(2.0))
                RS_SCALE = -0.5 * LN2 / (1 << 23)
                RS_BIAS = 0.5 * LN2 * (127 + 0.0450466) + 0.5 * float(np.log(HD))
                MINBITS = int(np.float32(1e-6).view(np.int32))
                rsb = cp.tile([128, 1], dt.float32, tag="rsb")
                nc.gpsimd.memset(rsb[:], RS_BIAS)

                def row_sq_ssq(raw, rowps, u):
                    """square + partition row-sums into row 32u of rowps
                    (matmul outputs may only start at partition 0/32/64)."""
                    sq = sp.tile([128, HW], dt.bfloat16, tag=f"sq{u}", bufs=1)
                    nc.scalar.square(sq[:], raw)
                    for col in range(HW // 512):
                        lo = 512 * col
                        nc.tensor.matmul(rowps[32 * u : 32 * u + 1, lo : lo + 512], lhsT=ones_col[:], rhs=sq[:, lo : lo + 512], start=True, stop=True)

                def rsq_chain(rowps):
                    """y2[32u,:] ~= rsqrt(rowps[32u,:]/HD), all rows at once.
                    In-place two-buffer Newton ladder on the DVE."""
                    A = sp.tile([128, HW], dt.float32, tag="rsA", bufs=1)
                    B = sp.tile([128, HW], dt.float32, tag="rsB", bufs=1)
                    nc.vector.tensor_scalar(A[:], rowps[:].bitcast(dt.int32), MINBITS, None, ALU.max)
                    nc.scalar.activation(B[:], A[:], AF.Exp, bias=rsb[:, 0:1], scale=RS_SCALE)
                    nc.vector.tensor_mul(A[:], B[:], B[:])
                    nc.vector.tensor_mul(A[:], A[:], rowps[:])
                    nc.vector.tensor_scalar(A[:], A[:], -0.5 / HD, 1.5, ALU.mult, ALU.add)
                    nc.vector.tensor_mul(A[:], B[:], A[:])  # A = y1
                    nc.vector.tensor_mul(B[:], A[:], A[:])
                    nc.vector.tensor_mul(B[:], B[:], rowps[:])
                    nc.vector.tensor_scalar(B[:], B[:], -0.5 / HD, 1.5, ALU.mult, ALU.add)
                    nc.vector.tensor_mul(B[:], A[:], B[:])  # B = y2
                    return B

                def row_epi_part2(raw, hs, y2, u):
                    """PE broadcast (with per-head gain^2/sqrt(HD) folded into
                    the q lhsT) + normalize + RoPE."""
                    lhs = growq[32 * u : 32 * u + 1, :]
                    qh = sp.tile([128, HW], dt.bfloat16, tag="qh")
                    for col in range(HW // 512):
                        lo = 512 * col
                        rb = rbp.tile([128, 512], dt.float32, tag="rb")
                        nc.tensor.matmul(rb[:], lhsT=lhs, rhs=y2[32 * u : 32 * u + 1, lo : lo + 512], start=True, stop=True)
                        nc.vector.tensor_mul(qh[:, lo : lo + 512], raw[:, lo : lo + 512], rb[:])
                    qsw = sp.tile([128, HW], dt.bfloat16, tag="qsw")
                    nc.vector.tensor_copy(qsw[0:64, :], qh[64:128, :])
                    nc.vector.tensor_copy(qsw[64:128, :], qh[0:64, :])
                    tsw = sp.tile([128, HW], dt.bfloat16, tag="tsw")
                    nc.vector.tensor_mul(tsw[:], qsw[:], sinF[:, hs : hs + HW])
                    tco = sp.tile([128, HW], dt.bfloat16, tag="tco")
                    nc.vector.tensor_mul(tco[:], qh[:], cosF[:, hs : hs + HW])
                    nc.vector.tensor_add(raw, tco[:], tsw[:])

                def vnat_transpose(grp):
                    vtp = vtrp.tile([128, 512], dt.bfloat16, tag="vtr")
                    for s_ in range(4):
                        kt = 4 * grp + s_
                        nc.tensor.transpose(vtp[:, 128 * s_ : 128 * (s_ + 1)], vT_sb[:, 128 * kt : 128 * (kt + 1)], ident[:])
                    nc.vector.tensor_copy(vnat[:, 512 * grp : 512 * (grp + 1)], vtp[:])

                # deferred work queue: callables interleaved into later bursts
                deferred = []

                GD = 4  # d-tiles per coalesced DMA group
                for hf in range(n_half):
                    hs = HW * hf
                    xg = []
                    stb = []
                    rowps = rowp.tile([128, HW], dt.float32, tag="rowps")
                    y2h = {}
                    raws = {}
                    # x first on BOTH rings (q units need only x), then skip
                    for gi in range(n_dt // GD):
                        xx = xp.tile([128, GD * HW], dt.bfloat16, tag="xg")
                        eng = nc.sync if gi % 2 == 0 else nc.scalar
                        eng.dma_start(
                            out=xx[:].rearrange("p (k f) -> p k f", k=GD),
                            in_=xT_d[128 * GD * gi : 128 * GD * (gi + 1), hs : hs + HW].rearrange("(k p) f -> p k f", p=128),
                        )
                        xg.append(xx)
                    if hf == 0:
                        late_weight_dmas()
                    for gi in range(n_dt // GD):
                        ss = skp.tile([128, GD * HW], dt.bfloat16, tag="sg")
                        eng = nc.scalar if gi % 2 == 0 else nc.sync
                        eng.dma_start(
                            out=ss[:].rearrange("p (k f) -> p k f", k=GD),
                            in_=skT_d[128 * GD * gi : 128 * GD * (gi + 1), hs : hs + HW].rearrange("(k p) f -> p k f", p=128),
                        )
                        for kk in range(GD):
                            bl = sbp.tile([128, HW], dt.bfloat16, tag="stb")
                            nc.vector.scalar_tensor_tensor(
                                bl[:], ss[:, kk * HW : (kk + 1) * HW], rbl128[:, 0:1],
                                xg[gi][:, kk * HW : (kk + 1) * HW], ALU.mult, ALU.add,
                            )
                            stb.append(bl)

                    def xs(k, ls):
                        return xg[k // GD][:, (k % GD) * HW + ls.start : (k % GD) * HW + ls.stop]

                    for kind in ("q0", "q1", "k", "v"):
                        for col in range(HW // 512):
                            # interleave one deferred epilogue ahead of the
                            # burst: its ACT chain drained a burst ago, so
                            # its PE ops slot in without stalling the FIFO
                            if deferred:
                                deferred.pop(0)()
                            cs = slice(hs + 512 * col, hs + 512 * (col + 1))
                            ls = slice(512 * col, 512 * (col + 1))
                            ps = psp.tile([128, 512], dt.float32, tag="proj_ps")
                            if kind == "k":
                                dest = kT[:, cs]
                                for k in range(n_dt):
                                    nc.tensor.matmul(ps[:], lhsT=wkb[:, k * HD : (k + 1) * HD], rhs=stb[k][:, ls], start=(k == 0), stop=(k == n_dt - 1))
                            elif kind == "v":
                                dest = vT_sb[:, cs]
                                for k in range(n_dt):
                                    nc.tensor.matmul(ps[:], lhsT=wvb[:, k * HD : (k + 1) * HD], rhs=stb[k][:, ls], start=(k == 0), stop=(k == n_dt - 1))
                            else:
                                h = int(kind[1])
                                dest = qT[:, t * h + hs + 512 * col : t * h + hs + 512 * (col + 1)]
                                for k in range(n_dt):
                                    nc.tensor.matmul(
                                        ps[:],
                                        lhsT=wq_sb[:, k * HQ * HD + h * HD : k * HQ * HD + (h + 1) * HD],
                                        rhs=xs(k, ls),
                                        start=(k == 0),
                                        stop=(k == n_dt - 1),
                                    )
                            if kind == "v":
                                # v epilogue inline: add (1-g)*v_bias
                                nc.vector.tensor_scalar_add(dest, ps[:], vbsT[:, 0:1])
                            else:
                                nc.scalar.activation(dest, ps[:], AF.Copy)
                        # queue each row's sq+rowsum as soon as its bursts are
                        # emitted; the rsqrt chain rides with k's (all 6 rows
                        # ready then); part2 broadcasts ride after v, always a
                        # burst behind so the ACT/DVE chains have drained.
                        # vnat transposes run inline after v.
                        if kind == "v":
                            vnat_transpose(2 * hf)
                            vnat_transpose(2 * hf + 1)
                            for u in range(3):
                                def p2(u=u, hs=hs, rowps=rowps, y2h=y2h, raws=raws):
                                    row_epi_part2(raws[u], hs, y2h[0], u)
                                deferred.append(p2)
                        else:
                            if kind == "k":
                                u = 2
                                raw = kT[:, hs : hs + HW]
                            else:
                                u = int(kind[1])
                                raw = qT[:, t * u + hs : t * u + hs + HW]
                            raws[u] = raw
                            if kind == "k":
                                def pk(raw=raw, rowps=rowps, y2h=y2h, u=u):
                                    row_sq_ssq(raw, rowps, u)
                                    y2h[0] = rsq_chain(rowps)
                                deferred.append(pk)
                            else:
                                def pq(raw=raw, rowps=rowps, u=u):
                                    row_sq_ssq(raw, rowps, u)
                                deferred.append(pq)

                # drain remaining deferred work
                for d in deferred:
                    d()
                deferred = []
            wraw.release()

            # ---- phase 2: attention ----
            y_in = [dp.tile([NCORES, HD, rows], dt.bfloat16, name=f"y_in{h}", tag=f"y_in{h}") for h in range(HQ)]
            y_out = [dp.tile([NCORES, HD, rows], dt.bfloat16, name=f"y_out{h}", tag=f"y_out{h}") for h in range(HQ)]

            # wproj prefetch (full, both HWDGE rings) during phase 2;
            # one 2 MB coalesced DMA per column block
            prp = tc.alloc_tile_pool(name="pr_s", bufs=4)
            wps = {}
            for n in range(D // 512):
                wp = prp.tile([128, n_dt * 512], dt.bfloat16, name=f"wp{n}", tag="wp")
                eng = nc.sync if n % 2 == 0 else nc.scalar
                eng.dma_start(
                    out=wp[:].rearrange("p (k f) -> p k f", k=n_dt),
                    in_=wpT_d[:, 512 * n : 512 * (n + 1)].rearrange("(k p) f -> p k f", p=128),
                )
                wps[n] = wp

            ytp_ = tc.alloc_tile_pool(name="yt_s", bufs=1)
            yt_blocks = [None] * n_dt
            with (
                tc.tile_pool(name="att_s", bufs=5) as ap_,
                tc.tile_pool(name="acc_s", bufs=2) as ap2,
                tc.tile_pool(name="st_ps", bufs=3, space="PSUM") as stp_,
                tc.tile_pool(name="yl_ps", bufs=2, space="PSUM") as ylp_,
            ):
                def epi_part1(h, c, ytp, accA, accB):
                    # l = ones.T @ (accA + accB); 1/l on the DVE (table-free)
                    lrow_t = stp_.tile([128, 1024], dt.float32, tag="st", name="lrow_t")
                    lrow = lrow_t[0:1, 0:512]
                    nc.tensor.matmul(lrow, lhsT=ones_col[:], rhs=accA[:], start=True, stop=False)
                    nc.tensor.matmul(lrow, lhsT=ones_col[:], rhs=accB[:], start=False, stop=True)
                    rl = ap_.tile([1, 512], dt.float32, tag="rl")
                    nc.vector.reciprocal_approx_fast(out=rl[:], in_=lrow)
                    return (h, c, ytp, rl)

                def epi_part2(h, c, ytp, rl):
                    # broadcast 1/l, normalize, ship pieces
                    rb2_t = stp_.tile([128, 1024], dt.float32, tag="st", name="rb2_t")
                    rb2 = rb2_t[:, 0:512]
                    nc.tensor.matmul(rb2, lhsT=onef_row[:], rhs=rl[:], start=True, stop=True)
                    rb2s = ap_.tile([128, 512], dt.float32, tag="rb2s")
                    nc.vector.tensor_copy(rb2s[:], rb2)
                    ysb = ap_.tile([128, 512], dt.bfloat16, tag="ysb")
                    nc.vector.tensor_mul(ysb[:], ytp[:], rb2s[:])
                    for b in range(512 // rows):
                        piece = (512 * c) // rows + b
                        nc.sync.dma_start(
                            out=y_in[h][piece, :, :],
                            in_=ysb[:, rows * b : rows * (b + 1)],
                        )

                prev_epi = None  # (h, c, ytp, acc): chunk awaiting part1
                epi1 = None  # (h, c, ytp, rl): awaiting part2
                for h in range(HQ):
                    for c in range(n_ch):
                        qs = slice(t * h + 512 * c, t * h + 512 * (c + 1))
                        nkts = kpc * (c + 1)
                        ytp = ylp_.tile([128, 512], dt.float32, tag="yt")
                        # two alternating accumulators halve the serial DVE
                        # dependency chain for the softmax denominator
                        accA = ap2.tile([128, 512], dt.bfloat16, tag="accA")
                        accB = ap2.tile([128, 512], dt.bfloat16, tag="accB")
                        pend = []  # [(pp, kts)] awaiting y/acc emission (2-deep)

                        def emit_pend(p, last, ytp=ytp, accA=accA, accB=accB):
                            ppp, kts_ = p
                            for s_, kt_ in enumerate(kts_):
                                pseg = ppp[:, 512 * s_ : 512 * (s_ + 1)]
                                nc.tensor.matmul(ytp[:], lhsT=vnat[:, HD * kt_ : HD * (kt_ + 1)], rhs=pseg, start=(kt_ == 0), stop=(last and kt_ == kts_[-1]))
                                acc = accA if kt_ % 2 == 0 else accB
                                if kt_ < 2:
                                    nc.vector.tensor_copy(acc[:], pseg)
                                else:
                                    nc.vector.tensor_add(acc[:], acc[:], pseg)

                        for pgi in range(nkts // 2):
                            kts = [2 * pgi, 2 * pgi + 1]
                            stp = stp_.tile([128, 1024], dt.float32, tag="st")
                            for s, kt in enumerate(kts):
                                seg = stp[:, 512 * s : 512 * (s + 1)]
                                diag = kt >= kpc * c
                                nc.tensor.matmul(seg, lhsT=kT[:, 128 * kt : 128 * (kt + 1)], rhs=qT[:, qs], start=True, stop=not diag)
                                if diag:
                                    m = kt - kpc * c
                                    nc.tensor.matmul(seg, lhsT=ident[:], rhs=mask[:, 512 * m : 512 * (m + 1)], start=False, stop=True)
                            # 2-deep lookahead: y/acc for group i-2 land after
                            # scores of group i, so exp latency is fully hidden
                            if len(pend) >= 2:
                                emit_pend(pend.pop(0), last=False)
                            pp = ap_.tile([128, 1024], dt.bfloat16, tag="pp")
                            nc.scalar.activation(pp[:], stp[:], AF.Exp, bias=negCmax[:, 0:1], scale=1.0)
                            npg = nkts // 2
                            p1_at = min(npg - 2, 3) if npg >= 3 else 0
                            if pgi == p1_at and prev_epi is not None:
                                epi1 = epi_part1(*prev_epi)
                                prev_epi = None
                            elif pgi == p1_at + 1 and epi1 is not None:
                                epi_part2(*epi1)
                                epi1 = None
                            pend.append((pp, kts))
                        while pend:
                            emit_pend(pend.pop(0), last=(len(pend) == 0))
                        prev_epi = (h, c, ytp, accA, accB)
                    # flush last chunk's epilogue before the collective
                    epi_part2(*epi_part1(*prev_epi))
                    prev_epi = None
                    nc.gpsimd.collective_compute(
                        "AllToAll",
                        ALU.bypass,
                        replica_groups=[list(range(NCORES))],
                        ins=[y_in[h].opt()],
                        outs=[y_out[h].opt()],
                    )
                    if h == 0:
                        yb = ytp_.tile([128, NCORES * rows], dt.bfloat16, name="ytall0", tag="ytall0")
                        nc.sync.dma_start(
                            out=yb[:].rearrange("p (j r) -> p j r", j=NCORES),
                            in_=y_out[0][:].rearrange("j p r -> p j r"),
                        )
                        for j in range(NCORES):
                            yt_blocks[2 * j] = yb[:, rows * j : rows * (j + 1)]

            # ---- phase 3: output projection (h0 pass overlaps A2A(h1)) ----
            yb1 = ytp_.tile([128, NCORES * rows], dt.bfloat16, name="ytall1", tag="ytall1")
            nc.sync.dma_start(
                out=yb1[:].rearrange("p (j r) -> p j r", j=NCORES),
                in_=y_out[1][:].rearrange("j p r -> p j r"),
            )
            for j in range(NCORES):
                yt_blocks[2 * j + 1] = yb1[:, rows * j : rows * (j + 1)]

            mb = min(128, rows)
            nb = rows // mb
            tiles3 = [(n, b) for n in range(D // 512) for b in range(nb)]
            with (
                tc.tile_pool(name="pr_ps", bufs=1, space="PSUM") as prps,
                tc.tile_pool(name="pr_out", bufs=2) as prout,
            ):
                opss = {}
                for (n, b) in tiles3:
                    ops = prps.tile([mb, 512], dt.float32, tag=f"ops{n}_{b}")
                    opss[(n, b)] = ops
                    for ai, a in enumerate(range(0, n_dt, 2)):  # h0 blocks
                        nc.tensor.matmul(
                            ops[:],
                            lhsT=yt_blocks[a][:, mb * b : mb * (b + 1)],
                            rhs=wps[n][:, 512 * a : 512 * (a + 1)],
                            start=(ai == 0),
                            stop=False,
                        )
                for (n, b) in tiles3:
                    ops = opss[(n, b)]
                    for ai, a in enumerate(range(1, n_dt, 2)):  # h1 blocks
                        nc.tensor.matmul(
                            ops[:],
                            lhsT=yt_blocks[a][:, mb * b : mb * (b + 1)],
                            rhs=wps[n][:, 512 * a : 512 * (a + 1)],
                            start=False,
                            stop=(ai == n_dt // 2 - 1),
                        )
                    osb = prout.tile([mb, 512], dt.bfloat16, tag="osb")
                    nc.scalar.activation(osb[:], ops[:], AF.Copy, scale=lns128[:mb, 0:1])
                    nc.sync.dma_start(
                        out=out_d[mb * b : mb * (b + 1), 512 * n : 512 * (n + 1)],
                        in_=osb[:],
                    )
            ytp_.release()
            prp.release()
    nc.finalize()
    return nc


def make_tables(t=T):
    pos = np.arange(t, dtype=np.float32)
    inv = 1.0 / (ROPE_BASE ** (np.arange(0, HD, 2, dtype=np.float32) / HD))
    fr = pos[:, None] * inv[None, :]  # [t, 64]
    cos = np.cos(fr).T  # [64, t]
    sin = np.sin(fr).T
    cosF = np.concatenate([cos, cos], axis=0)  # [128, t]
    sinF = np.concatenate([sin, -sin], axis=0)
    return _bf(cosF), _bf(sinF)


def make_masks():
    # mask[p, 512*m + j] = 0 if j >= 128*m + p else MASK_VAL
    p = np.arange(128)[:, None]
    j = np.arange(512)[None, :]
    blocks = [np.where(j >= 128 * m + p, 0.0, MASK_VAL) for m in range(4)]
    return _bf(np.concatenate(blocks, axis=1))


_GRAPH_CACHE = {}
_LAST_IN_MAPS = None


def kernel(x, skip, wq, wk, wv, wproj, qk_g, ln_s, v_bias):
    t = x.shape[1]
    if t not in _GRAPH_CACHE:
        _GRAPH_CACHE[t] = build_graph(t)
    nc = _GRAPH_CACHE[t]

    xT = _bf(x.reshape(t, D).T)
    skT = _bf(skip.reshape(t, D).T)
    wpT = _bf(np.asarray(wproj, np.float32).T)
    cosF, sinF = make_tables(t)
    masks = make_masks()
    ident = _bf(np.eye(128, dtype=np.float32))

    in_maps = []
    for c in range(NCORES):
        kv = c // 2
        in_maps.append(
            {
                "xT": xT,
                "skipT": skT,
                "wqT": _bf(np.asarray(wq, np.float32)[HQ * HD * c : HQ * HD * (c + 1), :].T),
                "wkT": _bf(np.asarray(wk, np.float32)[HD * kv : HD * (kv + 1), :].T),
                "wvT": _bf(np.asarray(wv, np.float32)[HD * kv : HD * (kv + 1), :].T),
                "wprojT": wpT,
                "qkg": np.asarray(qk_g, np.float32)[HQ * c : HQ * (c + 1)].reshape(1, HQ),
                "lns": np.asarray(ln_s, np.float32).reshape(1, 1),
                "vbias": np.asarray(v_bias, np.float32)[kv].reshape(1, HD),
                "cosF": cosF,
                "sinF": sinF,
                "masks": masks,
                "ident": ident,
            }
        )
    global _LAST_IN_MAPS
    _LAST_IN_MAPS = in_maps
    res = run_bass_kernel_spmd(nc, in_maps, list(range(NCORES)))
    out = np.concatenate(
        [np.asarray(res.results[c]["out"], np.float32) for c in range(NCORES)], axis=0
    )
    return out.reshape(1, t, D).astype(np.float32)

